# revision 31
# baseline (speedup 1.0000x reference)
"""TRN2 Bass kernel for nn_SynthesisLayer (StyleGAN-style modulated 3D conv).

Math: for each sample b
  styles = w[b] @ affine_weight.T / sqrt(512) + affine_bias          [Cin]
  wmod   = weight * styles[None,:,None]                              [Co,Ci,27]
  dcoef  = rsqrt(sum_{ci,k} wmod^2 + 1e-8)                           [Co]
  y      = dcoef * conv3d(x[b], wmod, pad=1) + noise_const*ns + bias
  out    = clip(lrelu(y)*sqrt(2), -256, 256)

Device implementation (per core): fp8 DoubleRow matmuls at 4x the fp32r
FLOP rate (2 K=128 tiles per instruction, 0.5 cycles/row). Accuracy is
recovered with a 3-term error-compensated split computed from host-side
e4m3 quantization:
  wmod ~= Wq + F8   (Wq = e4m3(wmod), F8 = e4m3(wmod - Wq))
  x    ~= Xq + E8   (Xq = e4m3(x),    E8 = e4m3(x - Xq))
  conv(wmod, x) ~= Wq*Xq + Wq*E8 + F8*Xq      (F8*E8 term negligible)
A few correction k-tiles are dropped (DROP_E/DROP_C) to trade a little
accuracy for PE time: 74 k-tiles per output chunk in 37 DoubleRow
matmuls vs 27 full-rate fp32r passes -> ~0.69x PE cycles of the fp32r
direct conv. Measured end-to-end device rel err ~1.4e-2 (gate 2e-2,
deterministic: device outputs are bit-exact across runs).

The conv itself is shifted matmuls (contraction over Cin=128 on
partitions). Outputs are computed per slice over the 32 real rows in
contiguous 352-position chunks (33-wide padded rows) so each tap
window is a contiguous run; moving APs stay at 2 free dims and the two
windows of a DR pair sit in different slots (pair stride must be EVEN
and windows disjoint or the device crashes). The host strips the pad
column afterwards. Demod/noise/bias/lrelu/clamp fold into a
per-partition scale (DVE stt + ACT Prelu + DVE clamp); noise is
DMA-broadcast per slice, pre-scaled by the host. Zero-weight warmup
matmuls burn the PE p-state ramp during the initial DMAs.

Sharding: 8 cores = 4 samples x 2 D-halves. Each core gets zero-padded
fp8 slabs [128, 2, NSLAB] (Xq, E8), computes output [128, 16896],
host reassembles. No collectives.
"""

import math
import os
import sys
import time

for _p in ("/opt/trn_rl_repo", "/root/.axon_site/_ro/trn_rl_repo"):
    if os.path.isdir(_p) and _p not in sys.path:
        sys.path.insert(0, _p)

import numpy as np
import ml_dtypes

import concourse.mybir as mybir
from concourse import bacc, bass
from concourse.tile import TileContext
from concourse.bass_utils import run_bass_kernel_spmd

P = 128          # Cin = Cout = 128
TAPS = 27        # 3x3x3
RES = 32
B = 4
W_DIM = 512
ROW = 33         # padded row width  (32 real + 1 zero)
SLICE = ROW * ROW  # 1089 padded slice (32 real rows + 1 zero row)
LEAD = 34        # leading zero guard (one row + one elem)
NSLICES = 18     # 16 output slices + 1 halo each side
BODY = NSLICES * SLICE
NSLAB = LEAD + BODY + 46   # tail guard
DHALF = 16                 # output D slices per core
RROWS = RES * ROW          # 1056 positions of the 32 real rows of a slice
NPOUT = DHALF * RROWS      # 16896 output positions per core (pad cols incl)
NCHUNK = 352               # chunk size: 3 chunks per slice of real rows
NPAIR = 37                 # DoubleRow matmuls per output chunk (74 k-tiles)
# Correction k-tiles dropped to cut PE time; each dropped tap adds
# ~sqrt(m)-scaled quantization error. Measured end-to-end device rel err
# with these 7 drops: ~1.5e-2 vs the 2e-2 gate (deterministic inputs).
DROP_E = (9, 11, 13, 15, 17)   # taps with no Wq*E8 correction
DROP_C = (10, 16)              # taps with no F8*Xq correction
LRELU_ALPHA = 0.2
LRELU_GAIN = math.sqrt(2.0)
CLAMP = 256.0

f32 = mybir.dt.float32
f8 = mybir.dt.float8e4
E4 = ml_dtypes.float8_e4m3
AF = mybir.ActivationFunctionType
DRMODE = mybir.MatmulPerfMode.DoubleRow

# tap k = 9*kd + 3*kh + kw reads input at (output position) + TAPOFF[k]
TAPOFF = [
    (kd - 1) * SLICE + (kh - 1) * ROW + (kw - 1)
    for kd in range(3) for kh in range(3) for kw in range(3)
]

# DoubleRow pair table. 81 k-tiles must be packed into 41 DR matmuls; the
# two moving windows of a DR matmul are (offset, offset+dq). Hardware
# constraint (found empirically): odd dq crashes the device, so dq must be
# EVEN; we additionally keep windows disjoint (dq >= 1021) and moderate
# (dq <= 2246), which holds under every stride rule consistent with the
# smoke tests. That forces pairing k-tiles of the SAME x slot across
# different kd planes (|dq| ~ SLICE or 2*SLICE, parity fixed via the
# in-plane offset delta).
# k-tiles: ('A',k)=Wq_k*Xq  ('C',k)=F8_k*Xq   [slot 0]
#          ('B',k)=Wq_k*E8                    [slot 1]
# 'Z' = zero weights (junk window partner for the odd tile out).


def _match_group(tiles):
    """Constructive pairing with dq = off_b - off_a even and in
    [1021, 2246]. Tap offset = 1089*(kd-1) + 33*(kh-1) + (kw-1), so dq is
    even iff kd+kh+kw parities match, and the magnitude window forces
    delta-kd of 1 or 2. Phase 1 pairs every kd=1 tile across delta-kd=1
    with opposite in-plane parity (kh+kw); phase 2 pairs kd=0 with kd=2
    at same in-plane parity. Returns (pairs, leftover)."""
    def kd(t):
        return t[0][1] // 9

    def q(t):
        k = t[0][1]
        return ((k // 3) % 3 + k % 3) % 2

    G = {(d, p): [t for t in tiles if kd(t) == d and q(t) == p]
         for d in range(3) for p in range(2)}
    pairs = []
    for p in (0, 1):
        mid = G[(1, p)]
        a, b = G[(0, 1 - p)], G[(2, 1 - p)]
        ha = len(mid) // 2
        for t in mid[:ha]:
            pairs.append((a.pop(), t))
        for t in mid[ha:]:
            pairs.append((t, b.pop()))
    leftover = None
    for p in (0, 1):
        a, b = G[(0, p)], G[(2, p)]
        while a and b:
            pairs.append((a.pop(), b.pop()))
        for t in a + b:
            assert leftover is None, "more than one leftover"
            leftover = t
    return pairs, leftover


def _build_pair_geom():
    slot0 = [(("A", k), TAPOFF[k]) for k in range(TAPS)]
    slot0 += [(("C", k), TAPOFF[k]) for k in range(TAPS) if k not in DROP_C]
    slot1 = [(("B", k), TAPOFF[k]) for k in range(TAPS) if k not in DROP_E]
    p0, left0 = _match_group(slot0)
    p1, left1 = _match_group(slot1)
    assert left0 is None and left1 is None
    geom = []   # (slot, off_first, dq, wkey_first, wkey_second)
    for (ka, offa), (kb, offb) in p0:
        geom.append((0, offa, offb - offa, ka, kb))
    for (ka, offa), (kb, offb) in p1:
        geom.append((1, offa, offb - offa, ka, kb))
    assert all(g[2] % 2 == 0 and 1021 <= g[2] <= 2246 for g in geom)
    assert len(geom) == NPAIR
    return geom


PAIR_GEOM = _build_pair_geom()

_NC_CACHE = None
LAST_EXEC_NS = None


def build_nc():
    nc = bacc.Bacc("TRN2", target_bir_lowering=False, debug=False, num_devices=8)

    xq2 = nc.dram_tensor("xq2", [P, 2, NSLAB], f8, kind="ExternalInput")
    wp = nc.dram_tensor("wp", [P, 2 * NPAIR, P], f8, kind="ExternalInput")
    scol = nc.dram_tensor("scol", [P, 1], f32, kind="ExternalInput")
    bcol = nc.dram_tensor("bcol", [P, 1], f32, kind="ExternalInput")
    acol = nc.dram_tensor("acol", [P, 1], f32, kind="ExternalInput")
    nz = nc.dram_tensor("nz", [1, NPOUT], f32, kind="ExternalInput")
    y = nc.dram_tensor("y", [P, NPOUT], f32, kind="ExternalOutput")

    with TileContext(nc) as tc:
        with (
            tc.tile_pool(name="big", bufs=1) as big,
            tc.tile_pool(name="small", bufs=1) as small,
            tc.tile_pool(name="nzp", bufs=4) as nzp,
            tc.tile_pool(name="outp", bufs=4) as outp,
            tc.tile_pool(name="wpsum", bufs=1, space="PSUM") as wpsum,
            tc.tile_pool(name="cpsum", bufs=7, space="PSUM") as cpsum,
        ):
            # ---- PE warmup: zero matmuls with no DMA deps, so the PE
            # p-state ramp (slow first ~3us) burns off while input DMAs run
            warmw = small.tile([P, 2, P], f8)
            warmx = small.tile([P, 2, NCHUNK], f8)
            nc.vector.memset(warmx[:], 0.0)
            nc.vector.memset(warmw[:], 0.0)
            wps = wpsum.tile([P, NCHUNK], f32, tag="warm")
            warm_rhs = bass.AP(
                tensor=warmx.tensor, offset=warmx.offset,
                ap=[warmx.ap[0], [NCHUNK, 2], [1, NCHUNK]],
            )
            for _ in range(18):
                nc.tensor.matmul(
                    wps[:], warmw[:], warm_rhs,
                    start=True, stop=True, perf_mode=DRMODE,
                )

            # ---- loads; ordered so the first conv chunk unblocks earliest:
            # first x pieces, first weight piece, consts, then the rest
            xq2_sb = big.tile([P, 2, NSLAB], f8)
            wp_sb = big.tile([P, 2 * NPAIR, P], f8)
            bounds = [0, LEAD + 3 * SLICE, LEAD + 6 * SLICE, LEAD + 9 * SLICE,
                      LEAD + 12 * SLICE, LEAD + 15 * SLICE, NSLAB]
            for s in range(2):
                nc.sync.dma_start(
                    xq2_sb[:, s, : bounds[1]], xq2[:, s, : bounds[1]]
                )
            for a, b_ in ((0, 24), (24, 48), (48, 2 * NPAIR)):
                nc.sync.dma_start(wp_sb[:, a:b_, :], wp[:, a:b_, :])
            scol_sb = small.tile([P, 1], f32)
            nc.sync.dma_start(scol_sb[:], scol[:])
            bcol_sb = small.tile([P, 1], f32)
            nc.sync.dma_start(bcol_sb[:], bcol[:])
            acol_sb = small.tile([P, 1], f32)
            nc.sync.dma_start(acol_sb[:], acol[:])
            for i in range(1, len(bounds) - 1):
                a, b_ = bounds[i], bounds[i + 1]
                for s in range(2):
                    nc.sync.dma_start(xq2_sb[:, s, a:b_], xq2[:, s, a:b_])

            def pair_rhs(j, p0, n):
                slot, off, dq, _, _ = PAIR_GEOM[j]
                return bass.AP(
                    tensor=xq2_sb.tensor,
                    offset=xq2_sb.offset + slot * NSLAB + p0 + off,
                    ap=[xq2_sb.ap[0], [dq, 2], [1, n]],
                )

            # ---- main conv loop: 3 chunks of 352 per output slice,
            # covering only the 32 real rows (pad rows never computed).
            # The final slice uses smaller trailing chunks so the end-of-
            # kernel postprocess+DMA drain is shorter.
            for d in range(DHALF):
                sbase = LEAD + (d + 1) * SLICE
                # noise pre-scaled by host (ns * sqrt(2)), one broadcast
                # DMA per slice
                nz_sl = nzp.tile([P, 1, RROWS], f32, tag="nz")
                nc.sync.dma_start(
                    nz_sl[:],
                    nz[:, d * RROWS : (d + 1) * RROWS].partition_broadcast(P),
                )
                if d < DHALF - 1:
                    parts = [(0, NCHUNK), (NCHUNK, NCHUNK), (2 * NCHUNK, NCHUNK)]
                else:
                    parts = [(0, NCHUNK), (NCHUNK, NCHUNK), (704, 264), (968, 88)]
                for po, clen in parts:
                    p0 = sbase + po
                    pt = cpsum.tile([P, NCHUNK], f32, tag="conv")
                    for j in range(NPAIR):
                        nc.tensor.matmul(
                            pt[:, :clen], wp_sb[:, 2 * j : 2 * j + 2, :],
                            pair_rhs(j, p0, clen),
                            start=(j == 0), stop=(j == NPAIR - 1),
                            perf_mode=DRMODE,
                        )
                    off = d * RROWS + po
                    ut = outp.tile([P, NCHUNK], f32, tag="out")
                    # ut = psum * (dcoef*sqrt2) + noise_term
                    nc.vector.scalar_tensor_tensor(
                        ut[:, :clen], pt[:, :clen], scol_sb[:],
                        nz_sl[:, 0, po : po + clen],
                        mybir.AluOpType.mult, mybir.AluOpType.add,
                    )
                    nc.scalar.activation(
                        ut[:, :clen], ut[:, :clen], AF.Prelu,
                        bias=bcol_sb[:], scale=1.0, alpha=acol_sb[:],
                    )
                    nc.vector.tensor_scalar(
                        ut[:, :clen], ut[:, :clen], CLAMP, -CLAMP,
                        mybir.AluOpType.min, mybir.AluOpType.max,
                    )
                    nc.sync.dma_start(y[:, off : off + clen], ut[:, :clen])

    nc.compile()
    return nc


def _get_nc():
    global _NC_CACHE
    if _NC_CACHE is None:
        _NC_CACHE = build_nc()
    return _NC_CACHE


def _make_core_inputs(x, w, affine_weight, affine_bias, weight, noise_const,
                      noise_strength, bias):
    """Build the 8 per-core input maps (host-side quantization + layout)."""
    styles = (w @ affine_weight.T) * (1.0 / math.sqrt(W_DIM)) + affine_bias
    wt_host = np.ascontiguousarray(
        weight.reshape(P, P, TAPS).transpose(1, 2, 0)
    )  # [ci, k, co]

    # full-volume fp8 split of x (shared by the two cores of each sample)
    xq_full = x.astype(E4)                                   # [B,P,32,32,32]
    e8_full = (x - xq_full.astype(np.float32)).astype(E4)

    acol_host = np.full((P, 1), LRELU_ALPHA, np.float32)
    bcol_host = (bias * LRELU_GAIN).reshape(P, 1).astype(np.float32)
    ns = float(noise_strength.reshape(-1)[0])
    nz_scaled = (noise_const * (ns * LRELU_GAIN)).astype(np.float32)

    # noise in output layout [16 slices][32 real rows][33 cols] per D-half
    nz_pad = np.zeros((2, DHALF, RES, ROW), np.float32)
    for half in range(2):
        nz_pad[half, :, :, :RES] = nz_scaled[half * DHALF:(half + 1) * DHALF]
    nz_pad = nz_pad.reshape(2, 1, NPOUT)

    in_maps = []
    per_sample = {}
    for c in range(8):
        b, half = divmod(c, 2)
        if b not in per_sample:
            wmod = wt_host * styles[b][:, None, None]        # [ci, k, co]
            dco = 1.0 / np.sqrt(
                (wmod.astype(np.float64) ** 2).sum(axis=(0, 1)) + 1e-8
            )  # [co]
            wq = wmod.astype(E4)
            f8q = (wmod - wq.astype(np.float32)).astype(E4)
            # DoubleRow lhsT pair layout [ci, 2*NPAIR, co]
            wmap = {"A": wq, "C": f8q, "B": wq}
            wp_host = np.zeros((P, 2 * NPAIR, P), E4)
            for j, (_, _, _, wka, wkb) in enumerate(PAIR_GEOM):
                for i, (t, k) in enumerate((wka, wkb)):
                    if t != "Z":
                        wp_host[:, 2 * j + i, :] = wmap[t][:, k, :]
            scol_host = (dco * LRELU_GAIN).reshape(P, 1).astype(np.float32)
            per_sample[b] = (wp_host, scol_host)
        wp_host, scol_host = per_sample[b]

        d0 = DHALF * half
        slab = np.zeros((P, 2, NSLAB), E4)
        view = slab[:, :, LEAD : LEAD + BODY].reshape(P, 2, NSLICES, ROW, ROW)
        lo = max(0, d0 - 1)
        hi = min(RES, d0 + DHALF + 1)
        # padded slice s holds global slice d0-1+s
        view[:, 0, lo - (d0 - 1) : hi - (d0 - 1), :RES, :RES] = xq_full[b, :, lo:hi]
        view[:, 1, lo - (d0 - 1) : hi - (d0 - 1), :RES, :RES] = e8_full[b, :, lo:hi]
        in_maps.append({
            "xq2": slab,
            "wp": wp_host,
            "scol": scol_host,
            "bcol": bcol_host,
            "acol": acol_host,
            "nz": np.ascontiguousarray(nz_pad[half]),
        })
    return in_maps


def kernel(x, w, affine_weight, affine_bias, weight, noise_const,
           noise_strength, bias):
    global LAST_EXEC_NS
    x = np.asarray(x, np.float32)
    w = np.asarray(w, np.float32)
    affine_weight = np.asarray(affine_weight, np.float32)
    affine_bias = np.asarray(affine_bias, np.float32)
    weight = np.asarray(weight, np.float32)
    noise_const = np.asarray(noise_const, np.float32)
    noise_strength = np.asarray(noise_strength, np.float32)
    bias = np.asarray(bias, np.float32)

    nc = _get_nc()
    in_maps = _make_core_inputs(
        x, w, affine_weight, affine_bias, weight, noise_const,
        noise_strength, bias,
    )
    trace = bool(os.environ.get("KERNEL_TRACE"))
    if trace:
        from concourse.bass_utils import axon_active

        if axon_active():
            try:  # axon NTFF capture needs the profile hook; absent in some pods
                from antenv.axon_hooks import get_axon_ntff_profile_hook  # noqa: F401
            except ImportError:
                trace = False
    res = None
    for attempt in range(3):
        try:
            res = run_bass_kernel_spmd(
                nc, in_maps, core_ids=list(range(8)), trace=trace
            )
            break
        except Exception:
            # transient NRT device wedge; retry after a short pause
            if attempt == 2:
                raise
            time.sleep(2.0)
    LAST_EXEC_NS = res.exec_time_ns

    out = np.empty((B, P, RES, RES, RES), np.float32)
    for c in range(8):
        b, half = divmod(c, 2)
        d0 = DHALF * half
        ypad = res.results[c]["y"].reshape(P, DHALF, RES, ROW)
        out[b, :, d0 : d0 + DHALF] = ypad[:, :, :, :RES]
    return out


# revision 32
# speedup vs baseline: 1.0011x; 1.0011x over previous
"""TRN2 Bass kernel for nn_SynthesisLayer (StyleGAN-style modulated 3D conv).

Math: for each sample b
  styles = w[b] @ affine_weight.T / sqrt(512) + affine_bias          [Cin]
  wmod   = weight * styles[None,:,None]                              [Co,Ci,27]
  dcoef  = rsqrt(sum_{ci,k} wmod^2 + 1e-8)                           [Co]
  y      = dcoef * conv3d(x[b], wmod, pad=1) + noise_const*ns + bias
  out    = clip(lrelu(y)*sqrt(2), -256, 256)

Device implementation (per core): fp8 DoubleRow matmuls at 4x the fp32r
FLOP rate (2 K=128 tiles per instruction, 0.5 cycles/row). Accuracy is
recovered with a 3-term error-compensated split computed from host-side
e4m3 quantization:
  wmod ~= Wq + F8   (Wq = e4m3(wmod), F8 = e4m3(wmod - Wq))
  x    ~= Xq + E8   (Xq = e4m3(x),    E8 = e4m3(x - Xq))
  conv(wmod, x) ~= Wq*Xq + Wq*E8 + F8*Xq      (F8*E8 term negligible)
A few correction k-tiles are dropped (DROP_E/DROP_C) to trade a little
accuracy for PE time: 74 k-tiles per output chunk in 37 DoubleRow
matmuls vs 27 full-rate fp32r passes -> ~0.69x PE cycles of the fp32r
direct conv. Measured end-to-end device rel err ~1.4e-2 (gate 2e-2,
deterministic: device outputs are bit-exact across runs).

The conv itself is shifted matmuls (contraction over Cin=128 on
partitions). Outputs are computed per slice over the 32 real rows in
contiguous 352-position chunks (33-wide padded rows) so each tap
window is a contiguous run; moving APs stay at 2 free dims and the two
windows of a DR pair sit in different slots (pair stride must be EVEN
and windows disjoint or the device crashes). The host strips the pad
column afterwards. Demod/noise/bias/lrelu/clamp fold into a
per-partition scale (DVE stt + ACT Prelu + DVE clamp); noise is
DMA-broadcast per slice, pre-scaled by the host. Zero-weight warmup
matmuls burn the PE p-state ramp during the initial DMAs.

Sharding: 8 cores = 4 samples x 2 D-halves. Each core gets zero-padded
fp8 slabs [128, 2, NSLAB] (Xq, E8), computes output [128, 16896],
host reassembles. No collectives.
"""

import math
import os
import sys
import time

for _p in ("/opt/trn_rl_repo", "/root/.axon_site/_ro/trn_rl_repo"):
    if os.path.isdir(_p) and _p not in sys.path:
        sys.path.insert(0, _p)

import numpy as np
import ml_dtypes

import concourse.mybir as mybir
from concourse import bacc, bass
from concourse.tile import TileContext
from concourse.bass_utils import run_bass_kernel_spmd

P = 128          # Cin = Cout = 128
TAPS = 27        # 3x3x3
RES = 32
B = 4
W_DIM = 512
ROW = 33         # padded row width  (32 real + 1 zero)
SLICE = ROW * ROW  # 1089 padded slice (32 real rows + 1 zero row)
LEAD = 34        # leading zero guard (one row + one elem)
NSLICES = 18     # 16 output slices + 1 halo each side
BODY = NSLICES * SLICE
NSLAB = LEAD + BODY + 46   # tail guard
DHALF = 16                 # output D slices per core
RROWS = RES * ROW          # 1056 positions of the 32 real rows of a slice
NPOUT = DHALF * RROWS      # 16896 output positions per core (pad cols incl)
NCHUNK = 352               # chunk size: 3 chunks per slice of real rows
NPAIR = 37                 # DoubleRow matmuls per output chunk (74 k-tiles)
# Correction k-tiles dropped to cut PE time; each dropped tap adds
# ~sqrt(m)-scaled quantization error. Measured end-to-end device rel err
# with these 7 drops: ~1.5e-2 vs the 2e-2 gate (deterministic inputs).
DROP_E = (9, 11, 13, 15, 17)   # taps with no Wq*E8 correction
DROP_C = (10, 16)              # taps with no F8*Xq correction
LRELU_ALPHA = 0.2
LRELU_GAIN = math.sqrt(2.0)
CLAMP = 256.0

f32 = mybir.dt.float32
f8 = mybir.dt.float8e4
E4 = ml_dtypes.float8_e4m3
AF = mybir.ActivationFunctionType
DRMODE = mybir.MatmulPerfMode.DoubleRow

# tap k = 9*kd + 3*kh + kw reads input at (output position) + TAPOFF[k]
TAPOFF = [
    (kd - 1) * SLICE + (kh - 1) * ROW + (kw - 1)
    for kd in range(3) for kh in range(3) for kw in range(3)
]

# DoubleRow pair table. 81 k-tiles must be packed into 41 DR matmuls; the
# two moving windows of a DR matmul are (offset, offset+dq). Hardware
# constraint (found empirically): odd dq crashes the device, so dq must be
# EVEN; we additionally keep windows disjoint (dq >= 1021) and moderate
# (dq <= 2246), which holds under every stride rule consistent with the
# smoke tests. That forces pairing k-tiles of the SAME x slot across
# different kd planes (|dq| ~ SLICE or 2*SLICE, parity fixed via the
# in-plane offset delta).
# k-tiles: ('A',k)=Wq_k*Xq  ('C',k)=F8_k*Xq   [slot 0]
#          ('B',k)=Wq_k*E8                    [slot 1]
# 'Z' = zero weights (junk window partner for the odd tile out).


def _match_group(tiles):
    """Constructive pairing with dq = off_b - off_a even and in
    [1021, 2246]. Tap offset = 1089*(kd-1) + 33*(kh-1) + (kw-1), so dq is
    even iff kd+kh+kw parities match, and the magnitude window forces
    delta-kd of 1 or 2. Phase 1 pairs every kd=1 tile across delta-kd=1
    with opposite in-plane parity (kh+kw); phase 2 pairs kd=0 with kd=2
    at same in-plane parity. Returns (pairs, leftover)."""
    def kd(t):
        return t[0][1] // 9

    def q(t):
        k = t[0][1]
        return ((k // 3) % 3 + k % 3) % 2

    G = {(d, p): [t for t in tiles if kd(t) == d and q(t) == p]
         for d in range(3) for p in range(2)}
    pairs = []
    for p in (0, 1):
        mid = G[(1, p)]
        a, b = G[(0, 1 - p)], G[(2, 1 - p)]
        ha = len(mid) // 2
        for t in mid[:ha]:
            pairs.append((a.pop(), t))
        for t in mid[ha:]:
            pairs.append((t, b.pop()))
    leftover = None
    for p in (0, 1):
        a, b = G[(0, p)], G[(2, p)]
        while a and b:
            pairs.append((a.pop(), b.pop()))
        for t in a + b:
            assert leftover is None, "more than one leftover"
            leftover = t
    return pairs, leftover


def _build_pair_geom():
    slot0 = [(("A", k), TAPOFF[k]) for k in range(TAPS)]
    slot0 += [(("C", k), TAPOFF[k]) for k in range(TAPS) if k not in DROP_C]
    slot1 = [(("B", k), TAPOFF[k]) for k in range(TAPS) if k not in DROP_E]
    p0, left0 = _match_group(slot0)
    p1, left1 = _match_group(slot1)
    assert left0 is None and left1 is None
    geom = []   # (slot, off_first, dq, wkey_first, wkey_second)
    for (ka, offa), (kb, offb) in p0:
        geom.append((0, offa, offb - offa, ka, kb))
    for (ka, offa), (kb, offb) in p1:
        geom.append((1, offa, offb - offa, ka, kb))
    assert all(g[2] % 2 == 0 and 1021 <= g[2] <= 2246 for g in geom)
    assert len(geom) == NPAIR
    return geom


PAIR_GEOM = _build_pair_geom()

_NC_CACHE = None
LAST_EXEC_NS = None


def build_nc():
    nc = bacc.Bacc("TRN2", target_bir_lowering=False, debug=False, num_devices=8)

    xq2 = nc.dram_tensor("xq2", [P, 2, NSLAB], f8, kind="ExternalInput")
    wp = nc.dram_tensor("wp", [P, 2 * NPAIR, P], f8, kind="ExternalInput")
    scol = nc.dram_tensor("scol", [P, 1], f32, kind="ExternalInput")
    bcol = nc.dram_tensor("bcol", [P, 1], f32, kind="ExternalInput")
    acol = nc.dram_tensor("acol", [P, 1], f32, kind="ExternalInput")
    nz = nc.dram_tensor("nz", [1, NPOUT], f32, kind="ExternalInput")
    y = nc.dram_tensor("y", [P, NPOUT], f32, kind="ExternalOutput")

    with TileContext(nc) as tc:
        with (
            tc.tile_pool(name="big", bufs=1) as big,
            tc.tile_pool(name="small", bufs=1) as small,
            tc.tile_pool(name="nzp", bufs=4) as nzp,
            tc.tile_pool(name="outp", bufs=4) as outp,
            tc.tile_pool(name="wpsum", bufs=1, space="PSUM") as wpsum,
            tc.tile_pool(name="cpsum", bufs=7, space="PSUM") as cpsum,
        ):
            # ---- PE warmup: zero matmuls with no DMA deps, so the PE
            # p-state ramp (slow first ~3us) burns off while input DMAs run
            warmw = small.tile([P, 2, P], f8)
            warmx = small.tile([P, 2, NCHUNK], f8)
            nc.vector.memset(warmx[:], 0.0)
            nc.vector.memset(warmw[:], 0.0)
            wps = wpsum.tile([P, NCHUNK], f32, tag="warm")
            warm_rhs = bass.AP(
                tensor=warmx.tensor, offset=warmx.offset,
                ap=[warmx.ap[0], [NCHUNK, 2], [1, NCHUNK]],
            )
            for _ in range(26):
                nc.tensor.matmul(
                    wps[:], warmw[:], warm_rhs,
                    start=True, stop=True, perf_mode=DRMODE,
                )

            # ---- loads; ordered so the first conv chunk unblocks earliest:
            # first x pieces, first weight piece, consts, then the rest
            xq2_sb = big.tile([P, 2, NSLAB], f8)
            wp_sb = big.tile([P, 2 * NPAIR, P], f8)
            bounds = [0, LEAD + 3 * SLICE, LEAD + 6 * SLICE, LEAD + 9 * SLICE,
                      LEAD + 12 * SLICE, LEAD + 15 * SLICE, NSLAB]
            for s in range(2):
                nc.sync.dma_start(
                    xq2_sb[:, s, : bounds[1]], xq2[:, s, : bounds[1]]
                )
            for a, b_ in ((0, 24), (24, 48), (48, 2 * NPAIR)):
                nc.sync.dma_start(wp_sb[:, a:b_, :], wp[:, a:b_, :])
            scol_sb = small.tile([P, 1], f32)
            nc.sync.dma_start(scol_sb[:], scol[:])
            bcol_sb = small.tile([P, 1], f32)
            nc.sync.dma_start(bcol_sb[:], bcol[:])
            acol_sb = small.tile([P, 1], f32)
            nc.sync.dma_start(acol_sb[:], acol[:])
            for i in range(1, len(bounds) - 1):
                a, b_ = bounds[i], bounds[i + 1]
                for s in range(2):
                    nc.sync.dma_start(xq2_sb[:, s, a:b_], xq2[:, s, a:b_])

            def pair_rhs(j, p0, n):
                slot, off, dq, _, _ = PAIR_GEOM[j]
                return bass.AP(
                    tensor=xq2_sb.tensor,
                    offset=xq2_sb.offset + slot * NSLAB + p0 + off,
                    ap=[xq2_sb.ap[0], [dq, 2], [1, n]],
                )

            # ---- main conv loop: 3 chunks of 352 per output slice,
            # covering only the 32 real rows (pad rows never computed).
            # The final slice uses smaller trailing chunks so the end-of-
            # kernel postprocess+DMA drain is shorter.
            for d in range(DHALF):
                sbase = LEAD + (d + 1) * SLICE
                # noise pre-scaled by host (ns * sqrt(2)), one broadcast
                # DMA per slice
                nz_sl = nzp.tile([P, 1, RROWS], f32, tag="nz")
                nc.sync.dma_start(
                    nz_sl[:],
                    nz[:, d * RROWS : (d + 1) * RROWS].partition_broadcast(P),
                )
                if d < DHALF - 1:
                    parts = [(0, NCHUNK), (NCHUNK, NCHUNK), (2 * NCHUNK, NCHUNK)]
                else:
                    parts = [(0, NCHUNK), (NCHUNK, NCHUNK), (704, 264), (968, 88)]
                for po, clen in parts:
                    p0 = sbase + po
                    pt = cpsum.tile([P, NCHUNK], f32, tag="conv")
                    for j in range(NPAIR):
                        nc.tensor.matmul(
                            pt[:, :clen], wp_sb[:, 2 * j : 2 * j + 2, :],
                            pair_rhs(j, p0, clen),
                            start=(j == 0), stop=(j == NPAIR - 1),
                            perf_mode=DRMODE,
                        )
                    off = d * RROWS + po
                    ut = outp.tile([P, NCHUNK], f32, tag="out")
                    # ut = psum * (dcoef*sqrt2) + noise_term
                    nc.vector.scalar_tensor_tensor(
                        ut[:, :clen], pt[:, :clen], scol_sb[:],
                        nz_sl[:, 0, po : po + clen],
                        mybir.AluOpType.mult, mybir.AluOpType.add,
                    )
                    nc.scalar.activation(
                        ut[:, :clen], ut[:, :clen], AF.Prelu,
                        bias=bcol_sb[:], scale=1.0, alpha=acol_sb[:],
                    )
                    nc.vector.tensor_scalar(
                        ut[:, :clen], ut[:, :clen], CLAMP, -CLAMP,
                        mybir.AluOpType.min, mybir.AluOpType.max,
                    )
                    nc.sync.dma_start(y[:, off : off + clen], ut[:, :clen])

    nc.compile()
    return nc


def _get_nc():
    global _NC_CACHE
    if _NC_CACHE is None:
        _NC_CACHE = build_nc()
    return _NC_CACHE


def _make_core_inputs(x, w, affine_weight, affine_bias, weight, noise_const,
                      noise_strength, bias):
    """Build the 8 per-core input maps (host-side quantization + layout)."""
    styles = (w @ affine_weight.T) * (1.0 / math.sqrt(W_DIM)) + affine_bias
    wt_host = np.ascontiguousarray(
        weight.reshape(P, P, TAPS).transpose(1, 2, 0)
    )  # [ci, k, co]

    # full-volume fp8 split of x (shared by the two cores of each sample)
    xq_full = x.astype(E4)                                   # [B,P,32,32,32]
    e8_full = (x - xq_full.astype(np.float32)).astype(E4)

    acol_host = np.full((P, 1), LRELU_ALPHA, np.float32)
    bcol_host = (bias * LRELU_GAIN).reshape(P, 1).astype(np.float32)
    ns = float(noise_strength.reshape(-1)[0])
    nz_scaled = (noise_const * (ns * LRELU_GAIN)).astype(np.float32)

    # noise in output layout [16 slices][32 real rows][33 cols] per D-half
    nz_pad = np.zeros((2, DHALF, RES, ROW), np.float32)
    for half in range(2):
        nz_pad[half, :, :, :RES] = nz_scaled[half * DHALF:(half + 1) * DHALF]
    nz_pad = nz_pad.reshape(2, 1, NPOUT)

    in_maps = []
    per_sample = {}
    for c in range(8):
        b, half = divmod(c, 2)
        if b not in per_sample:
            wmod = wt_host * styles[b][:, None, None]        # [ci, k, co]
            dco = 1.0 / np.sqrt(
                (wmod.astype(np.float64) ** 2).sum(axis=(0, 1)) + 1e-8
            )  # [co]
            wq = wmod.astype(E4)
            f8q = (wmod - wq.astype(np.float32)).astype(E4)
            # DoubleRow lhsT pair layout [ci, 2*NPAIR, co]
            wmap = {"A": wq, "C": f8q, "B": wq}
            wp_host = np.zeros((P, 2 * NPAIR, P), E4)
            for j, (_, _, _, wka, wkb) in enumerate(PAIR_GEOM):
                for i, (t, k) in enumerate((wka, wkb)):
                    if t != "Z":
                        wp_host[:, 2 * j + i, :] = wmap[t][:, k, :]
            scol_host = (dco * LRELU_GAIN).reshape(P, 1).astype(np.float32)
            per_sample[b] = (wp_host, scol_host)
        wp_host, scol_host = per_sample[b]

        d0 = DHALF * half
        slab = np.zeros((P, 2, NSLAB), E4)
        view = slab[:, :, LEAD : LEAD + BODY].reshape(P, 2, NSLICES, ROW, ROW)
        lo = max(0, d0 - 1)
        hi = min(RES, d0 + DHALF + 1)
        # padded slice s holds global slice d0-1+s
        view[:, 0, lo - (d0 - 1) : hi - (d0 - 1), :RES, :RES] = xq_full[b, :, lo:hi]
        view[:, 1, lo - (d0 - 1) : hi - (d0 - 1), :RES, :RES] = e8_full[b, :, lo:hi]
        in_maps.append({
            "xq2": slab,
            "wp": wp_host,
            "scol": scol_host,
            "bcol": bcol_host,
            "acol": acol_host,
            "nz": np.ascontiguousarray(nz_pad[half]),
        })
    return in_maps


def kernel(x, w, affine_weight, affine_bias, weight, noise_const,
           noise_strength, bias):
    global LAST_EXEC_NS
    x = np.asarray(x, np.float32)
    w = np.asarray(w, np.float32)
    affine_weight = np.asarray(affine_weight, np.float32)
    affine_bias = np.asarray(affine_bias, np.float32)
    weight = np.asarray(weight, np.float32)
    noise_const = np.asarray(noise_const, np.float32)
    noise_strength = np.asarray(noise_strength, np.float32)
    bias = np.asarray(bias, np.float32)

    nc = _get_nc()
    in_maps = _make_core_inputs(
        x, w, affine_weight, affine_bias, weight, noise_const,
        noise_strength, bias,
    )
    trace = bool(os.environ.get("KERNEL_TRACE"))
    if trace:
        from concourse.bass_utils import axon_active

        if axon_active():
            try:  # axon NTFF capture needs the profile hook; absent in some pods
                from antenv.axon_hooks import get_axon_ntff_profile_hook  # noqa: F401
            except ImportError:
                trace = False
    res = None
    for attempt in range(3):
        try:
            res = run_bass_kernel_spmd(
                nc, in_maps, core_ids=list(range(8)), trace=trace
            )
            break
        except Exception:
            # transient NRT device wedge; retry after a short pause
            if attempt == 2:
                raise
            time.sleep(2.0)
    LAST_EXEC_NS = res.exec_time_ns

    out = np.empty((B, P, RES, RES, RES), np.float32)
    for c in range(8):
        b, half = divmod(c, 2)
        d0 = DHALF * half
        ypad = res.results[c]["y"].reshape(P, DHALF, RES, ROW)
        out[b, :, d0 : d0 + DHALF] = ypad[:, :, :, :RES]
    return out


# revision 33
# speedup vs baseline: 1.0024x; 1.0014x over previous
"""TRN2 Bass kernel for nn_SynthesisLayer (StyleGAN-style modulated 3D conv).

Math: for each sample b
  styles = w[b] @ affine_weight.T / sqrt(512) + affine_bias          [Cin]
  wmod   = weight * styles[None,:,None]                              [Co,Ci,27]
  dcoef  = rsqrt(sum_{ci,k} wmod^2 + 1e-8)                           [Co]
  y      = dcoef * conv3d(x[b], wmod, pad=1) + noise_const*ns + bias
  out    = clip(lrelu(y)*sqrt(2), -256, 256)

Device implementation (per core): fp8 DoubleRow matmuls at 4x the fp32r
FLOP rate (2 K=128 tiles per instruction, 0.5 cycles/row). Accuracy is
recovered with a 3-term error-compensated split computed from host-side
e4m3 quantization:
  wmod ~= Wq + F8   (Wq = e4m3(wmod), F8 = e4m3(wmod - Wq))
  x    ~= Xq + E8   (Xq = e4m3(x),    E8 = e4m3(x - Xq))
  conv(wmod, x) ~= Wq*Xq + Wq*E8 + F8*Xq      (F8*E8 term negligible)
A few correction k-tiles are dropped (DROP_E/DROP_C) to trade a little
accuracy for PE time: 74 k-tiles per output chunk in 37 DoubleRow
matmuls vs 27 full-rate fp32r passes -> ~0.69x PE cycles of the fp32r
direct conv. Measured end-to-end device rel err ~1.4e-2 (gate 2e-2,
deterministic: device outputs are bit-exact across runs).

The conv itself is shifted matmuls (contraction over Cin=128 on
partitions). Outputs are computed per slice over the 32 real rows in
contiguous 352-position chunks (33-wide padded rows) so each tap
window is a contiguous run; moving APs stay at 2 free dims and the two
windows of a DR pair sit in different slots (pair stride must be EVEN
and windows disjoint or the device crashes). The host strips the pad
column afterwards. Demod/noise/bias/lrelu/clamp fold into a
per-partition scale (DVE stt + ACT Prelu + DVE clamp); noise is
DMA-broadcast per slice, pre-scaled by the host. Zero-weight warmup
matmuls burn the PE p-state ramp during the initial DMAs.

Sharding: 8 cores = 4 samples x 2 D-halves. Each core gets zero-padded
fp8 slabs [128, 2, NSLAB] (Xq, E8), computes output [128, 16896],
host reassembles. No collectives.
"""

import math
import os
import sys
import time

for _p in ("/opt/trn_rl_repo", "/root/.axon_site/_ro/trn_rl_repo"):
    if os.path.isdir(_p) and _p not in sys.path:
        sys.path.insert(0, _p)

import numpy as np
import ml_dtypes

import concourse.mybir as mybir
from concourse import bacc, bass
from concourse.tile import TileContext
from concourse.bass_utils import run_bass_kernel_spmd

P = 128          # Cin = Cout = 128
TAPS = 27        # 3x3x3
RES = 32
B = 4
W_DIM = 512
ROW = 33         # padded row width  (32 real + 1 zero)
SLICE = ROW * ROW  # 1089 padded slice (32 real rows + 1 zero row)
LEAD = 34        # leading zero guard (one row + one elem)
NSLICES = 18     # 16 output slices + 1 halo each side
BODY = NSLICES * SLICE
NSLAB = LEAD + BODY + 46   # tail guard
DHALF = 16                 # output D slices per core
RROWS = RES * ROW          # 1056 positions of the 32 real rows of a slice
NPOUT = DHALF * RROWS      # 16896 output positions per core (pad cols incl)
NCHUNK = 352               # chunk size: 3 chunks per slice of real rows
NPAIR = 37                 # DoubleRow matmuls per output chunk (74 k-tiles)
# Correction k-tiles dropped to cut PE time; each dropped tap adds
# ~sqrt(m)-scaled quantization error. Measured end-to-end device rel err
# with these 7 drops: ~1.5e-2 vs the 2e-2 gate (deterministic inputs).
DROP_E = (9, 11, 13, 15, 17)   # taps with no Wq*E8 correction
DROP_C = (10, 16)              # taps with no F8*Xq correction
LRELU_ALPHA = 0.2
LRELU_GAIN = math.sqrt(2.0)
CLAMP = 256.0

f32 = mybir.dt.float32
f8 = mybir.dt.float8e4
E4 = ml_dtypes.float8_e4m3
AF = mybir.ActivationFunctionType
DRMODE = mybir.MatmulPerfMode.DoubleRow

# tap k = 9*kd + 3*kh + kw reads input at (output position) + TAPOFF[k]
TAPOFF = [
    (kd - 1) * SLICE + (kh - 1) * ROW + (kw - 1)
    for kd in range(3) for kh in range(3) for kw in range(3)
]

# DoubleRow pair table. 81 k-tiles must be packed into 41 DR matmuls; the
# two moving windows of a DR matmul are (offset, offset+dq). Hardware
# constraint (found empirically): odd dq crashes the device, so dq must be
# EVEN; we additionally keep windows disjoint (dq >= 1021) and moderate
# (dq <= 2246), which holds under every stride rule consistent with the
# smoke tests. That forces pairing k-tiles of the SAME x slot across
# different kd planes (|dq| ~ SLICE or 2*SLICE, parity fixed via the
# in-plane offset delta).
# k-tiles: ('A',k)=Wq_k*Xq  ('C',k)=F8_k*Xq   [slot 0]
#          ('B',k)=Wq_k*E8                    [slot 1]
# 'Z' = zero weights (junk window partner for the odd tile out).


def _match_group(tiles):
    """Constructive pairing with dq = off_b - off_a even and in
    [1021, 2246]. Tap offset = 1089*(kd-1) + 33*(kh-1) + (kw-1), so dq is
    even iff kd+kh+kw parities match, and the magnitude window forces
    delta-kd of 1 or 2. Phase 1 pairs every kd=1 tile across delta-kd=1
    with opposite in-plane parity (kh+kw); phase 2 pairs kd=0 with kd=2
    at same in-plane parity. Returns (pairs, leftover)."""
    def kd(t):
        return t[0][1] // 9

    def q(t):
        k = t[0][1]
        return ((k // 3) % 3 + k % 3) % 2

    G = {(d, p): [t for t in tiles if kd(t) == d and q(t) == p]
         for d in range(3) for p in range(2)}
    pairs = []
    for p in (0, 1):
        mid = G[(1, p)]
        a, b = G[(0, 1 - p)], G[(2, 1 - p)]
        ha = len(mid) // 2
        for t in mid[:ha]:
            pairs.append((a.pop(), t))
        for t in mid[ha:]:
            pairs.append((t, b.pop()))
    leftover = None
    for p in (0, 1):
        a, b = G[(0, p)], G[(2, p)]
        while a and b:
            pairs.append((a.pop(), b.pop()))
        for t in a + b:
            assert leftover is None, "more than one leftover"
            leftover = t
    return pairs, leftover


def _build_pair_geom():
    slot0 = [(("A", k), TAPOFF[k]) for k in range(TAPS)]
    slot0 += [(("C", k), TAPOFF[k]) for k in range(TAPS) if k not in DROP_C]
    slot1 = [(("B", k), TAPOFF[k]) for k in range(TAPS) if k not in DROP_E]
    p0, left0 = _match_group(slot0)
    p1, left1 = _match_group(slot1)
    assert left0 is None and left1 is None
    geom = []   # (slot, off_first, dq, wkey_first, wkey_second)
    for (ka, offa), (kb, offb) in p0:
        geom.append((0, offa, offb - offa, ka, kb))
    for (ka, offa), (kb, offb) in p1:
        geom.append((1, offa, offb - offa, ka, kb))
    assert all(g[2] % 2 == 0 and 1021 <= g[2] <= 2246 for g in geom)
    assert len(geom) == NPAIR
    return geom


PAIR_GEOM = _build_pair_geom()

_NC_CACHE = None
LAST_EXEC_NS = None


def build_nc():
    nc = bacc.Bacc("TRN2", target_bir_lowering=False, debug=False, num_devices=8)

    xq2 = nc.dram_tensor("xq2", [P, 2, NSLAB], f8, kind="ExternalInput")
    wp = nc.dram_tensor("wp", [P, 2 * NPAIR, P], f8, kind="ExternalInput")
    scol = nc.dram_tensor("scol", [P, 1], f32, kind="ExternalInput")
    bcol = nc.dram_tensor("bcol", [P, 1], f32, kind="ExternalInput")
    acol = nc.dram_tensor("acol", [P, 1], f32, kind="ExternalInput")
    nz = nc.dram_tensor("nz", [1, NPOUT], f32, kind="ExternalInput")
    y = nc.dram_tensor("y", [P, NPOUT], f32, kind="ExternalOutput")

    with TileContext(nc) as tc:
        with (
            tc.tile_pool(name="big", bufs=1) as big,
            tc.tile_pool(name="small", bufs=1) as small,
            tc.tile_pool(name="nzp", bufs=4) as nzp,
            tc.tile_pool(name="outp", bufs=4) as outp,
            tc.tile_pool(name="wpsum", bufs=1, space="PSUM") as wpsum,
            tc.tile_pool(name="cpsum", bufs=7, space="PSUM") as cpsum,
        ):
            # ---- PE warmup: zero matmuls with no DMA deps, so the PE
            # p-state ramp (slow first ~3us) burns off while input DMAs run
            warmw = small.tile([P, 2, P], f8)
            nc.vector.memset(warmw[:], 0.0)
            warmx = small.tile([P, 2, NCHUNK], f8)
            nc.vector.memset(warmx[:], 0.0)
            wps = wpsum.tile([P, NCHUNK], f32, tag="warm")
            warm_rhs = bass.AP(
                tensor=warmx.tensor, offset=warmx.offset,
                ap=[warmx.ap[0], [NCHUNK, 2], [1, NCHUNK]],
            )
            for _ in range(26):
                nc.tensor.matmul(
                    wps[:], warmw[:], warm_rhs,
                    start=True, stop=True, perf_mode=DRMODE,
                )

            # ---- loads; ordered so the first conv chunk unblocks earliest:
            # first x pieces, first weight piece, consts, then the rest
            xq2_sb = big.tile([P, 2, NSLAB], f8)
            wp_sb = big.tile([P, 2 * NPAIR, P], f8)
            bounds = [0, LEAD + 3 * SLICE, LEAD + 6 * SLICE, LEAD + 9 * SLICE,
                      LEAD + 12 * SLICE, LEAD + 15 * SLICE, NSLAB]
            for s in range(2):
                nc.sync.dma_start(
                    xq2_sb[:, s, : bounds[1]], xq2[:, s, : bounds[1]]
                )
            for a, b_ in ((0, 24), (24, 48), (48, 2 * NPAIR)):
                nc.sync.dma_start(wp_sb[:, a:b_, :], wp[:, a:b_, :])
            scol_sb = small.tile([P, 1], f32)
            nc.sync.dma_start(scol_sb[:], scol[:])
            bcol_sb = small.tile([P, 1], f32)
            nc.sync.dma_start(bcol_sb[:], bcol[:])
            acol_sb = small.tile([P, 1], f32)
            nc.sync.dma_start(acol_sb[:], acol[:])
            for i in range(1, len(bounds) - 1):
                a, b_ = bounds[i], bounds[i + 1]
                for s in range(2):
                    nc.sync.dma_start(xq2_sb[:, s, a:b_], xq2[:, s, a:b_])

            def pair_rhs(j, p0, n):
                slot, off, dq, _, _ = PAIR_GEOM[j]
                return bass.AP(
                    tensor=xq2_sb.tensor,
                    offset=xq2_sb.offset + slot * NSLAB + p0 + off,
                    ap=[xq2_sb.ap[0], [dq, 2], [1, n]],
                )

            # ---- main conv loop: 3 chunks of 352 per output slice,
            # covering only the 32 real rows (pad rows never computed).
            # The final slice uses smaller trailing chunks so the end-of-
            # kernel postprocess+DMA drain is shorter.
            for d in range(DHALF):
                sbase = LEAD + (d + 1) * SLICE
                # noise pre-scaled by host (ns * sqrt(2)), one broadcast
                # DMA per slice
                nz_sl = nzp.tile([P, 1, RROWS], f32, tag="nz")
                nc.sync.dma_start(
                    nz_sl[:],
                    nz[:, d * RROWS : (d + 1) * RROWS].partition_broadcast(P),
                )
                if d < DHALF - 1:
                    parts = [(0, NCHUNK), (NCHUNK, NCHUNK), (2 * NCHUNK, NCHUNK)]
                else:
                    parts = [(0, NCHUNK), (NCHUNK, NCHUNK), (704, 176), (880, 176)]
                for po, clen in parts:
                    p0 = sbase + po
                    pt = cpsum.tile([P, NCHUNK], f32, tag="conv")
                    for j in range(NPAIR):
                        nc.tensor.matmul(
                            pt[:, :clen], wp_sb[:, 2 * j : 2 * j + 2, :],
                            pair_rhs(j, p0, clen),
                            start=(j == 0), stop=(j == NPAIR - 1),
                            perf_mode=DRMODE,
                        )
                    off = d * RROWS + po
                    ut = outp.tile([P, NCHUNK], f32, tag="out")
                    # ut = psum * (dcoef*sqrt2) + noise_term
                    nc.vector.scalar_tensor_tensor(
                        ut[:, :clen], pt[:, :clen], scol_sb[:],
                        nz_sl[:, 0, po : po + clen],
                        mybir.AluOpType.mult, mybir.AluOpType.add,
                    )
                    nc.scalar.activation(
                        ut[:, :clen], ut[:, :clen], AF.Prelu,
                        bias=bcol_sb[:], scale=1.0, alpha=acol_sb[:],
                    )
                    nc.vector.tensor_scalar(
                        ut[:, :clen], ut[:, :clen], CLAMP, -CLAMP,
                        mybir.AluOpType.min, mybir.AluOpType.max,
                    )
                    nc.sync.dma_start(y[:, off : off + clen], ut[:, :clen])

    nc.compile()
    return nc


def _get_nc():
    global _NC_CACHE
    if _NC_CACHE is None:
        _NC_CACHE = build_nc()
    return _NC_CACHE


def _make_core_inputs(x, w, affine_weight, affine_bias, weight, noise_const,
                      noise_strength, bias):
    """Build the 8 per-core input maps (host-side quantization + layout)."""
    styles = (w @ affine_weight.T) * (1.0 / math.sqrt(W_DIM)) + affine_bias
    wt_host = np.ascontiguousarray(
        weight.reshape(P, P, TAPS).transpose(1, 2, 0)
    )  # [ci, k, co]

    # full-volume fp8 split of x (shared by the two cores of each sample)
    xq_full = x.astype(E4)                                   # [B,P,32,32,32]
    e8_full = (x - xq_full.astype(np.float32)).astype(E4)

    acol_host = np.full((P, 1), LRELU_ALPHA, np.float32)
    bcol_host = (bias * LRELU_GAIN).reshape(P, 1).astype(np.float32)
    ns = float(noise_strength.reshape(-1)[0])
    nz_scaled = (noise_const * (ns * LRELU_GAIN)).astype(np.float32)

    # noise in output layout [16 slices][32 real rows][33 cols] per D-half
    nz_pad = np.zeros((2, DHALF, RES, ROW), np.float32)
    for half in range(2):
        nz_pad[half, :, :, :RES] = nz_scaled[half * DHALF:(half + 1) * DHALF]
    nz_pad = nz_pad.reshape(2, 1, NPOUT)

    in_maps = []
    per_sample = {}
    for c in range(8):
        b, half = divmod(c, 2)
        if b not in per_sample:
            wmod = wt_host * styles[b][:, None, None]        # [ci, k, co]
            dco = 1.0 / np.sqrt(
                (wmod.astype(np.float64) ** 2).sum(axis=(0, 1)) + 1e-8
            )  # [co]
            wq = wmod.astype(E4)
            f8q = (wmod - wq.astype(np.float32)).astype(E4)
            # DoubleRow lhsT pair layout [ci, 2*NPAIR, co]
            wmap = {"A": wq, "C": f8q, "B": wq}
            wp_host = np.zeros((P, 2 * NPAIR, P), E4)
            for j, (_, _, _, wka, wkb) in enumerate(PAIR_GEOM):
                for i, (t, k) in enumerate((wka, wkb)):
                    if t != "Z":
                        wp_host[:, 2 * j + i, :] = wmap[t][:, k, :]
            scol_host = (dco * LRELU_GAIN).reshape(P, 1).astype(np.float32)
            per_sample[b] = (wp_host, scol_host)
        wp_host, scol_host = per_sample[b]

        d0 = DHALF * half
        slab = np.zeros((P, 2, NSLAB), E4)
        view = slab[:, :, LEAD : LEAD + BODY].reshape(P, 2, NSLICES, ROW, ROW)
        lo = max(0, d0 - 1)
        hi = min(RES, d0 + DHALF + 1)
        # padded slice s holds global slice d0-1+s
        view[:, 0, lo - (d0 - 1) : hi - (d0 - 1), :RES, :RES] = xq_full[b, :, lo:hi]
        view[:, 1, lo - (d0 - 1) : hi - (d0 - 1), :RES, :RES] = e8_full[b, :, lo:hi]
        in_maps.append({
            "xq2": slab,
            "wp": wp_host,
            "scol": scol_host,
            "bcol": bcol_host,
            "acol": acol_host,
            "nz": np.ascontiguousarray(nz_pad[half]),
        })
    return in_maps


def kernel(x, w, affine_weight, affine_bias, weight, noise_const,
           noise_strength, bias):
    global LAST_EXEC_NS
    x = np.asarray(x, np.float32)
    w = np.asarray(w, np.float32)
    affine_weight = np.asarray(affine_weight, np.float32)
    affine_bias = np.asarray(affine_bias, np.float32)
    weight = np.asarray(weight, np.float32)
    noise_const = np.asarray(noise_const, np.float32)
    noise_strength = np.asarray(noise_strength, np.float32)
    bias = np.asarray(bias, np.float32)

    nc = _get_nc()
    in_maps = _make_core_inputs(
        x, w, affine_weight, affine_bias, weight, noise_const,
        noise_strength, bias,
    )
    trace = bool(os.environ.get("KERNEL_TRACE"))
    if trace:
        from concourse.bass_utils import axon_active

        if axon_active():
            try:  # axon NTFF capture needs the profile hook; absent in some pods
                from antenv.axon_hooks import get_axon_ntff_profile_hook  # noqa: F401
            except ImportError:
                trace = False
    res = None
    for attempt in range(3):
        try:
            res = run_bass_kernel_spmd(
                nc, in_maps, core_ids=list(range(8)), trace=trace
            )
            break
        except Exception:
            # transient NRT device wedge; retry after a short pause
            if attempt == 2:
                raise
            time.sleep(2.0)
    LAST_EXEC_NS = res.exec_time_ns

    out = np.empty((B, P, RES, RES, RES), np.float32)
    for c in range(8):
        b, half = divmod(c, 2)
        d0 = DHALF * half
        ypad = res.results[c]["y"].reshape(P, DHALF, RES, ROW)
        out[b, :, d0 : d0 + DHALF] = ypad[:, :, :, :RES]
    return out
